# revision 25
# baseline (speedup 1.0000x reference)
import numpy as np
import ml_dtypes
from contextlib import ExitStack

import jax
import concourse.bass as bass  # noqa
import concourse.mybir as mybir
import concourse.tile as tile
from concourse import bacc

B, C, H, W = 64, 3, 512, 512
NCORES = 8
HL = 66                # input rows per core (64 + 2 halo)
HO = 64                # output rows per core
NTS = B - 2            # 62 valid time steps (batch trim)
A = 255.0 / 64.0                       # DT/(2*DX)
M_ = 1e-5 * (1.0 / 32.0) * 255.0**2    # MU*DT/DX**2

LAST_EXEC_NS = None

BF16_MATS = ("D1", "Da", "L4", "Sp", "Sn", "Sm", "Sa", "San")


def _build_mats():
    """All 9 stationary matrices stacked into one [9*HL, HO] bf16 array
    (Ia zero-padded to HL rows) so they ship as a single input."""
    z = lambda: np.zeros((HL, HO), np.float32)
    D1, Da, L4, Sp, Sn, Sm, Sa, San, Ia = (z() for _ in range(9))
    for i in range(HO):
        D1[i + 2, i] = 1.0
        D1[i, i] = -1.0
        Da[i + 2, i] = A
        Da[i, i] = -A
        L4[i, i] = -M_
        L4[i + 1, i] = 4.0 * M_
        L4[i + 2, i] = -M_
        Sp[i + 1, i] = 1.0
        Sn[i + 1, i] = -1.0
        Sm[i + 1, i] = -M_
        Sa[i + 1, i] = A
        San[i + 1, i] = -A
        Ia[i, i] = A
    stack = np.concatenate([D1, Da, L4, Sp, Sn, Sm, Sa, San, Ia], axis=0)
    return stack.astype(ml_dtypes.bfloat16)


def _emit_blockslab(nc, M, Uc, Vc, Pc, Up, Un, Vp, Vn, Ucs, Vcs,
                    psA, psB, psR0, psDUX, psDVX, qs, std, outs):
    # Padded tiles: data column j lives at position j+1; positions 0 and 513
    # are ghosts. All MM dests are full-width offset-0; shifts are expressed
    # on the source side.
    Abs = mybir.ActivationFunctionType.Abs
    mm = nc.tensor.matmul
    CEN = slice(1, 513)
    RSH = slice(2, 514)
    LSH = slice(0, 512)

    mm(psDUX[0:HO, :], M["D1"], Uc[:, CEN], start=True, stop=True)
    mm(psDVX[0:HO, :], M["D1"], Vc[:, CEN], start=True, stop=True)
    mm(psR0[0:HO, :], M["D1"], Uc[:, CEN], start=True, stop=False)

    mm(psA[0:HO, :], M["Sp"], Un[:, CEN], start=True, stop=False)
    mm(psB[0:HO, :], M["Sp"], Vn[:, CEN], start=True, stop=False)
    mm(psR0[0:HO, :], M["Sp"], Vc[:, RSH], start=False, stop=False)

    mm(psA[0:HO, :], M["Sn"], Up[:, CEN], start=False, stop=False)
    mm(psB[0:HO, :], M["Sn"], Vp[:, CEN], start=False, stop=False)
    mm(psR0[0:HO, :], M["Sn"], Vc[:, LSH], start=False, stop=True)

    nc.scalar.activation(outs[0][0:HO, 1:511], psR0[0:HO, 1:511], Abs)

    mm(psA[0:HO, :], M["L4"], Uc[:, CEN], start=False, stop=False)
    mm(psB[0:HO, :], M["L4"], Vc[:, CEN], start=False, stop=False)

    mm(psA[0:HO, :], M["Da"], Pc[:, CEN], start=False, stop=False)

    mm(psB[0:HO, :], M["Sa"], Pc[:, RSH], start=False, stop=False)
    mm(psB[0:HO, :], M["San"], Pc[:, LSH], start=False, stop=False)

    mm(psA[0:HO, :], M["Sm"], Uc[:, RSH], start=False, stop=False)
    mm(psA[0:HO, :], M["Sm"], Uc[:, LSH], start=False, stop=False)
    mm(psB[0:HO, :], M["Sm"], Vc[:, RSH], start=False, stop=False)
    mm(psB[0:HO, :], M["Sm"], Vc[:, LSH], start=False, stop=False)

    dudy, dvdy, q1, q2, q3, q4 = qs
    sub = mybir.AluOpType.subtract
    mul = mybir.AluOpType.mult
    # Ucs/Vcs are partition-realigned padded copies of the center slab
    # (out row j <-> window row j+1; data col w at padded position w+1).
    nc.gpsimd.tensor_tensor(out=dudy[:, 1:511], in0=Ucs[:, 3:513],
                            in1=Ucs[:, 1:511], op=sub)
    nc.gpsimd.tensor_tensor(out=dvdy[:, 1:511], in0=Vcs[:, 3:513],
                            in1=Vcs[:, 1:511], op=sub)
    nc.vector.scalar_tensor_tensor(out=q1[:, 2:512], in0=Ucs[:, 2:512],
                                   scalar=std, in1=psDUX[0:HO, 1:511],
                                   op0=mul, op1=mul)
    nc.vector.scalar_tensor_tensor(out=q2[:, 2:512], in0=Vcs[:, 2:512],
                                   scalar=std, in1=dudy[:, 1:511],
                                   op0=mul, op1=mul)
    nc.vector.scalar_tensor_tensor(out=q3[:, 2:512], in0=Ucs[:, 2:512],
                                   scalar=std, in1=psDVX[0:HO, 1:511],
                                   op0=mul, op1=mul)
    nc.vector.scalar_tensor_tensor(out=q4[:, 2:512], in0=Vcs[:, 2:512],
                                   scalar=std, in1=dvdy[:, 1:511],
                                   op0=mul, op1=mul)

    mm(psA[0:HO, :], M["Ia"], q1[:, 1:513], start=False, stop=False)
    mm(psA[0:HO, :], M["Ia"], q2[:, 1:513], start=False, stop=True)
    mm(psB[0:HO, :], M["Ia"], q3[:, 1:513], start=False, stop=False)
    mm(psB[0:HO, :], M["Ia"], q4[:, 1:513], start=False, stop=True)

    nc.scalar.activation(outs[1][0:HO, 1:511], psA[0:HO, 1:511], Abs)
    nc.scalar.activation(outs[2][0:HO, 1:511], psB[0:HO, 1:511], Abs)


def _build_program(std):
    f32r = mybir.dt.float32r
    f32 = mybir.dt.float32
    bf16 = mybir.dt.bfloat16
    f8 = mybir.dt.float8e4
    Square = mybir.ActivationFunctionType.Square
    sub = mybir.AluOpType.subtract

    nc = bacc.Bacc("TRN2", target_bir_lowering=False, debug=False)
    xd = nc.dram_tensor("xl", [B, C, HL, W], f8, kind="ExternalInput")
    yd = nc.dram_tensor("yl", [B, C, HL, W], f8, kind="ExternalInput")
    mats_d = nc.dram_tensor("MS", [9 * HL, HO], bf16, kind="ExternalInput")
    acc_d = nc.dram_tensor("acc", [HO, NTS * 3], f32, kind="ExternalOutput")

    with ExitStack() as ctx:
        tc = ctx.enter_context(tile.TileContext(nc))
        mpool = ctx.enter_context(tc.tile_pool(name="mats", bufs=1))
        wpool = ctx.enter_context(tc.tile_pool(name="win", bufs=4))
        s8pool = ctx.enter_context(tc.tile_pool(name="st8", bufs=4))
        xapool = ctx.enter_context(tc.tile_pool(name="absx", bufs=2))
        qpool = ctx.enter_context(tc.tile_pool(name="q", bufs=2))
        spool = ctx.enter_context(tc.tile_pool(name="scr", bufs=2))
        cpool = ctx.enter_context(tc.tile_pool(name="cen", bufs=4))
        apool = ctx.enter_context(tc.tile_pool(name="accp", bufs=1))
        pab = ctx.enter_context(tc.tile_pool(name="psab", bufs=2, space="PSUM"))
        prx = ctx.enter_context(tc.tile_pool(name="psrx", bufs=2, space="PSUM"))

        M = {}
        for i, n in enumerate(BF16_MATS + ("Ia",)):
            rows = HL if n != "Ia" else HO
            t = mpool.tile([rows, HO], bf16, name=f"m_{n}")
            nc.sync.dma_start(out=t, in_=mats_d[i * HL:i * HL + rows, :])
            M[n] = t

        acc_s = apool.tile([HO, NTS * 3], f32, name="accs")

        # Rolling slab windows, keyed (src, field) -> {slab: tile}. `cen`
        # holds partition-realigned center copies (row j <-> slab row j+1),
        # loaded straight from DRAM: compute engines cannot read with a
        # partition offset and partition-shifted SBUF->SBUF DMA corrupts
        # data at this geometry.
        win = {("x", f): {} for f in "uvp"}
        win.update({("y", f): {} for f in "uvp"})
        cen = {("x", f): {} for f in "uv"}
        cen.update({("y", f): {} for f in "uv"})
        convs = {("x", "u"): nc.scalar, ("x", "v"): nc.gpsimd,
                 ("x", "p"): nc.vector, ("y", "u"): nc.scalar,
                 ("y", "v"): nc.gpsimd, ("y", "p"): nc.vector}

        def conv(eng, out, in_):
            if eng is nc.scalar:
                eng.copy(out=out, in_=in_)
            else:
                eng.tensor_scalar_mul(out=out, in0=in_, scalar1=1.0)

        def load_slab(tag, srcd, s):
            for fi, f in enumerate("uvp"):
                st = s8pool.tile([HL, W], f8, name=f"s8_{tag}{f}")
                nc.sync.dma_start(out=st, in_=srcd[s, fi, :, :])
                w = wpool.tile([HL, 514], bf16, name=f"w_{tag}{f}")
                conv(convs[(tag, f)], w[:, 1:513], st)
                win[(tag, f)][s] = w
                if f != "p":
                    st2 = s8pool.tile([HO, W], f8, name=f"c8_{tag}{f}")
                    nc.sync.dma_start(out=st2, in_=srcd[s, fi, 1:65, :])
                    wc = cpool.tile([HO, 514], bf16, name=f"wc_{tag}{f}")
                    conv(convs[(tag, f)], wc[:, 1:513], st2)
                    cen[(tag, f)][s] = wc
            for d in win.values():
                d.pop(s - 4, None)
            for d in cen.values():
                d.pop(s - 4, None)

        for s in (0, 1, 2):
            load_slab("x", xd, s)
            load_slab("y", yd, s)

        for t in range(1, B - 1):
            if t + 2 < B:
                load_slab("x", xd, t + 2)
                load_slab("y", yd, t + 2)

            ax = None
            for tag, srcd in (("x", xd), ("y", yd)):
                U, V, P = win[(tag, "u")], win[(tag, "v")], win[(tag, "p")]
                Ucs = cen[(tag, "u")][t]
                Vcs = cen[(tag, "v")][t]
                psA = pab.tile([HO, W], f32, name="psA")
                psB = pab.tile([HO, W], f32, name="psB")
                psR0 = prx.tile([HO, W], f32, name="psR0", bufs=1)
                psDUX = prx.tile([HO, W], f32, name="psDUX")
                psDVX = prx.tile([HO, W], f32, name="psDVX", bufs=1)
                qs = (
                    qpool.tile([HO, W], f32, name="dudy"),
                    qpool.tile([HO, W], f32, name="dvdy"),
                    qpool.tile([HO, 514], bf16, name="q1"),
                    qpool.tile([HO, 514], bf16, name="q2"),
                    qpool.tile([HO, 514], bf16, name="q3"),
                    qpool.tile([HO, 514], bf16, name="q4"),
                )
                if tag == "x":
                    outs = tuple(xapool.tile([HO, W], f32, name=f"ax{r}")
                                 for r in range(3))
                    ax = outs
                else:
                    outs = tuple(spool.tile([HO, W], f32, name=f"rT{r}")
                                 for r in range(3))
                _emit_blockslab(nc, M, U[t], V[t], P[t],
                                U[t - 1], U[t + 1], V[t - 1], V[t + 1],
                                Ucs, Vcs,
                                psA, psB, psR0, psDUX, psDVX, qs, std, outs)
                if tag == "y":
                    for r in range(3):
                        dif = spool.tile([HO, W], f32, name="dif")
                        nc.gpsimd.tensor_tensor(
                            out=dif[0:HO, 1:511],
                            in0=outs[r][0:HO, 1:511],
                            in1=ax[r][0:HO, 1:511], op=sub)
                        sqs = spool.tile([HO, W], f32, name="sqs")
                        col = (t - 1) * 3 + r
                        nc.scalar.activation(
                            sqs[0:HO, 1:511], dif[0:HO, 1:511], Square,
                            accum_out=acc_s[0:HO, col:col + 1])

        nc.sync.dma_start(out=acc_d[:, :], in_=acc_s)

    nc.finalize()
    return nc


_PROG_CACHE = {}


def _prof(tag, t0):
    import os, time
    if os.environ.get("BASSK_PROF"):
        print(f"  [prof] {tag}: {time.perf_counter() - t0:.3f}s", flush=True)
    return time.perf_counter()


def _run_overlapped(nc, make_chunks, small_inputs):
    """Execute `nc` on 8 cores. Large-input shard transfers are issued
    core-by-core as `make_chunks` produces them (asynchronously), so host
    conversion, jit trace + NEFF compile all overlap the tunnel streaming."""
    from concourse.bass2jax import (
        _bass_exec_p,
        install_neuronx_cc_hook,
        partition_id_tensor,
    )
    from jax.experimental.shard_map import shard_map
    from jax.sharding import Mesh, PartitionSpec, NamedSharding

    install_neuronx_cc_hook()

    small_inputs = dict(small_inputs)
    if nc.dbg_addr is not None:
        assert not nc.dbg_callbacks
        small_inputs[nc.dbg_addr.name] = np.zeros((1, 2), np.uint32)

    partition_name = nc.partition_id_tensor.name if nc.partition_id_tensor else None
    in_names, out_names, out_avals, zero_outs = [], [], [], []
    for alloc in nc.m.functions[0].allocations:
        if not isinstance(alloc, mybir.MemoryLocationSet):
            continue
        name = alloc.memorylocations[0].name
        if alloc.kind == "ExternalInput":
            if name != partition_name:
                in_names.append(name)
        elif alloc.kind == "ExternalOutput":
            shape = tuple(alloc.tensor_shape)
            dtype = mybir.dt.np(alloc.dtype)
            out_names.append(name)
            out_avals.append(jax.core.ShapedArray(shape, dtype))
            zero_outs.append(np.zeros(shape, dtype))
    n_params = len(in_names)
    n_outs = len(out_avals)
    in_names_all = list(in_names) + list(out_names)
    if partition_name is not None:
        in_names_all.append(partition_name)
    donate = tuple(range(n_params, n_params + n_outs))

    devs = jax.devices()[:NCORES]
    mesh = Mesh(np.asarray(devs), ("core",))
    sh = NamedSharding(mesh, PartitionSpec("core"))

    def gassemble(shards):
        gs = (sum(s.shape[0] for s in shards),) + tuple(shards[0].shape[1:])
        return jax.make_array_from_single_device_arrays(gs, sh, shards)

    def _body(*args):
        operands = list(args)
        if partition_name is not None:
            operands.append(partition_id_tensor())
        outs = _bass_exec_p.bind(
            *operands,
            out_avals=tuple(out_avals),
            in_names=tuple(in_names_all),
            out_names=tuple(out_names),
            lowering_input_output_aliases=(),
            sim_require_finite=True,
            sim_require_nnan=True,
            nc=nc,
        )
        return tuple(outs)

    fn = jax.jit(
        shard_map(
            _body,
            mesh=mesh,
            in_specs=(PartitionSpec("core"),) * (n_params + n_outs),
            out_specs=(PartitionSpec("core"),) * n_outs,
            check_rep=False,
        ),
        donate_argnums=donate,
        keep_unused=True,
    )

    # AOT-compile in a background thread (the heavy BIR->NEFF step is a
    # subprocess, so it runs concurrently) while the main thread converts
    # and uploads the input shards over the tunnel.
    shapes_by_name = {}
    for name in in_names:
        if name in make_chunks:
            probe = make_chunks[name](0)
            shapes_by_name[name] = ((NCORES * probe.shape[0],) + probe.shape[1:],
                                    probe.dtype)
            shapes_by_name[name + "__c0"] = probe
        else:
            arr = small_inputs[name]
            shapes_by_name[name] = ((NCORES * arr.shape[0],) + arr.shape[1:],
                                    arr.dtype)
    aot_args = [jax.ShapeDtypeStruct(*shapes_by_name[n], sharding=sh)
                for n in in_names]
    aot_args += [jax.ShapeDtypeStruct((NCORES * z.shape[0],) + z.shape[1:],
                                      z.dtype, sharding=sh) for z in zero_outs]
    compiled_box = {}

    def _compile():
        compiled_box["exe"] = fn.lower(*aot_args).compile()

    import time
    tp = time.perf_counter()
    import threading
    th = threading.Thread(target=_compile)
    th.start()
    tp = _prof("compile-thread-start", tp)

    # Interleave host-side chunk production with async shard uploads: the
    # tunnel streams chunk c while the host converts/slices chunk c+1.
    big_shards = {name: [] for name in make_chunks}
    for c in range(NCORES):
        for name, make in make_chunks.items():
            chunk = shapes_by_name.pop(name + "__c0", None) if c == 0 else None
            if chunk is None:
                chunk = make(c)
            big_shards[name].append(jax.device_put(chunk, devs[c]))
    tp = _prof("make+put all chunks", tp)

    globals_by_name = {n: gassemble(s) for n, s in big_shards.items()}
    for name, arr in small_inputs.items():
        globals_by_name[name] = gassemble(
            [jax.device_put(arr, d) for d in devs])

    global_args = [globals_by_name[n] for n in in_names]
    global_args += [gassemble([jax.device_put(z, d) for d in devs])
                    for z in zero_outs]
    tp = _prof("assemble+small puts", tp)

    th.join()
    tp = _prof("compile join", tp)
    out_arrs = compiled_box["exe"](*global_args)
    tp = _prof("dispatch", tp)
    res = {
        name: np.asarray(out_arrs[i]).reshape(NCORES, *out_avals[i].shape)
        for i, name in enumerate(out_names)
    }
    _prof("block+fetch", tp)
    return res


def kernel(x, y, std):
    global LAST_EXEC_NS
    stdf = float(std)
    if stdf not in _PROG_CACHE:
        _PROG_CACHE[stdf] = _build_program(stdf)
    nc = _PROG_CACHE[stdf]

    mats = _build_mats()

    def make(src, c):
        r0 = HO * c
        if c < 7:
            sl = src[:, :, r0:r0 + HL, :]
        else:
            sl = np.concatenate([src[:, :, r0:, :], src[:, :, -2:, :]], axis=2)
        return sl.astype(ml_dtypes.float8_e4m3)

    make_chunks = {"xl": lambda c: make(x, c), "yl": lambda c: make(y, c)}

    import time
    t0 = time.perf_counter_ns()
    res = _run_overlapped(nc, make_chunks, {"MS": mats})
    LAST_EXEC_NS = time.perf_counter_ns() - t0

    Nt = NTS * 510 * 510
    sc0 = (stdf * 127.5) ** 2
    sc12 = (32.0 * stdf) ** 2
    tot = 0.0
    for c in range(NCORES):
        acc = res["acc"][c].astype(np.float64)
        nrow = 62 if c == 7 else 64
        v = acc[:nrow].sum(axis=0).reshape(NTS, 3).sum(axis=0)
        tot += sc0 * v[0] + sc12 * (v[1] + v[2])
    return np.float32(0.001 * tot / Nt)
